# revision 17
# baseline (speedup 1.0000x reference)
"""DiagonalLinear: y = x * w + b (elementwise over features).

x: (16384, 4096) f32, w/b: (4096,) f32.
Sharding: data-parallel over batch across 8 NeuronCores (2048 rows each),
weight/bias replicated.

Per core: HWDGE loads x row-tiles [128, 4096] on the SP ring, DVE computes
mul+add in place, HWDGE stores on the ACT ring. The w/b partition-broadcast
DMAs ride the ACT ring (idle at start) so the first x-load completes early;
the first mul/add carry the const-DMA waits directly (bacc splits excess
waits onto EventSemaphores), which lets DVE start as soon as x-tile 0 and
the w broadcast land instead of serializing behind all const traffic.
"""

import numpy as np

import concourse.bacc as bacc
import concourse.bass as bass
import concourse.mybir as mybir
import concourse.tile as tile
from concourse.bass_utils import run_bass_kernel_spmd

N_CORES = 8
BATCH = 16384
D = 4096
ROWS_PER_CORE = BATCH // N_CORES  # 2048
P = 128

BUFS = 4

_CACHE = {}


def build_nc(bufs=BUFS):
    nc = bacc.Bacc()
    f32 = mybir.dt.float32
    x = nc.dram_tensor("x", [ROWS_PER_CORE, D], f32, kind="ExternalInput")
    wb_in = nc.dram_tensor("wb", [1, 2 * D], f32, kind="ExternalInput")
    y = nc.dram_tensor("y", [ROWS_PER_CORE, D], f32, kind="ExternalOutput")

    n_tiles = ROWS_PER_CORE // P  # 16
    x_r = x.rearrange("(n p) d -> n p d", p=P)
    y_r = y.rearrange("(n p) d -> n p d", p=P)

    with tile.TileContext(nc) as tc:
        with (
            tc.tile_pool(name="consts", bufs=1) as cpool,
            tc.tile_pool(name="work", bufs=bufs) as pool,
        ):
            consts = cpool.tile([P, 2 * D], f32)  # [:, :D]=w, [:, D:]=b
            wt = consts[:, 0:D]
            bt = consts[:, D : 2 * D]
            # First emission: x-tile 0 load, then the two const broadcasts on
            # the (empty) ACT ring. mul#0 then waits only {load0, w}; add#0
            # waits b, which lands just as mul#0 finishes.
            t0 = pool.tile([P, D], f32)
            nc.sync.dma_start(t0[:, :], x_r[0])
            nc.scalar.dma_start(wt, wb_in[:, 0:D].partition_broadcast(P))
            nc.scalar.dma_start(bt, wb_in[:, D : 2 * D].partition_broadcast(P))

            for i in range(n_tiles):
                t = t0 if i == 0 else pool.tile([P, D], f32)
                if i > 0:
                    nc.sync.dma_start(t[:, :], x_r[i])
                nc.vector.tensor_mul(t[:, :], t[:, :], wt)
                nc.vector.tensor_add(t[:, :], t[:, :], bt)
                nc.scalar.dma_start(y_r[i], t[:, :])
    nc.compile()
    return nc


def _get_nc():
    if "nc" not in _CACHE:
        _CACHE["nc"] = build_nc()
    return _CACHE["nc"]


def run(input, weight, bias, nc=None, **spmd_kwargs):
    if nc is None:
        nc = _get_nc()
    x = np.ascontiguousarray(input, dtype=np.float32)
    wb = np.ascontiguousarray(
        np.stack([np.asarray(weight), np.asarray(bias)]).astype(np.float32)
    ).reshape(1, 2 * D)
    in_maps = [
        {"x": x[c * ROWS_PER_CORE : (c + 1) * ROWS_PER_CORE], "wb": wb}
        for c in range(N_CORES)
    ]
    res = run_bass_kernel_spmd(nc, in_maps, core_ids=list(range(N_CORES)), **spmd_kwargs)
    out = np.concatenate([r["y"] for r in res.results], axis=0)
    return out, res


def kernel(input, weight, bias):
    out, _ = run(input, weight, bias)
    return out
